# revision 26
# baseline (speedup 1.0000x reference)
"""Attention1D Trainium2 Bass kernel, sharded over 8 NeuronCores.

Reference computation (per batch b, C=512 channels, T=2048, H=8 heads, d=64):
    qkv = qkv_w @ x + qkv_b            # [3C, T]
    q, k, v = split(qkv)               # each [C, T], viewed as H heads of d=64
    attn = softmax((q_h . k_h) * C**-0.5, over s)
    out_h = attn @ v_h
    out = proj_w @ concat(out_h) + proj_b
    result = x + out

Sharding: 8 cores = 4 batches x 2 head-groups (4 heads each).  Each core
computes its group's partial projection output y_g = proj_w[:, g] @ attn_g;
the host combines: out[b] = x[b] + (proj_b + proj_w @ v_bias) + y_0 + y_1.
(The v bias commutes through softmax-weighted averaging because the
probabilities sum to 1, so it folds into an effective projection bias.)

v2 design (bottleneck: ActE exp at ~156us busy in v1, PE ~154us):
  * Because the softmax scale is C**-0.5 (full channel count), logits are
    tiny (~N(0, 0.35)), so E = exp(logit) is in [~0.1, ~8] - comfortably
    inside fp8e4m3 range.  E and v^T are stored fp8 and the A.V matmul runs
    in fp8 DoubleRow mode (2 s-chunks contracted per instruction at 0.5
    cycles/row), halving its PE cost vs bf16.
  * The exp work is split between ActE (exact Exp activation, fp8 out) and
    the otherwise-idle Vector engine, which computes exp with a Schraudolph
    bit-trick: bits8 = rint(logit * 8/ln2 + (56 - 0.37)) interpreted as
    fp8e4m3 (one fused tensor_scalar mult+add with int8 output; hw rounds
    rint, verified).  ~3% rms error on attention weights, which cancels in
    the softmax normalization and averages out in A.V.
  * Softmax denominators: ones-columns packed into the DR stationary give
    Z rows inside the A.V PSUM accumulators for free.  Normalize path:
    ActE copies av PSUM -> SBUF bf16, SBUF->SBUF DMA gathers Z rows into
    numerator-aligned partitions, ActE exp(-ln Z) gives reciprocals, DVE
    bf16 2x-mode muls write the normalized attention output.
"""

import sys

if "/opt/trn_rl_repo" not in sys.path:
    sys.path.insert(0, "/opt/trn_rl_repo")

import numpy as np

try:
    import ml_dtypes

    _BF16 = ml_dtypes.bfloat16
    _FP8 = ml_dtypes.float8_e4m3
except ImportError:  # pragma: no cover
    _BF16 = None
    _FP8 = None

B, C, T, H = 4, 512, 2048, 8
D = C // H  # 64 head dim
G = 2  # head groups (cores per batch)
CG = C // G  # 256 channels per group
SCALE = float(C) ** -0.5
N_CORES = 8

# Schraudolph exp-as-int8 constants (fp8e4m3 bits; hw rint rounding).
SCH_A8 = 8.0 / float(np.log(2.0))
SCH_B8 = 7.0 * 8.0 - 0.37
# Fraction of the 128 exp tiles handled by ActE (exact); rest on DVE.
ACT_EXP_TILES = 62

_CACHE: dict = {}


def _build_nc():
    import concourse.tile as tile
    from concourse import bacc, mybir
    from concourse import hw_specs

    # Our only ScalarE table functions are Exp and Ln.  The table-load pass
    # picks the first set (by canonical act_info.json index) containing each
    # function, which would alternate between the exp-only set and the
    # ln+exp set and reload tables at every normalization.  Keep the
    # canonical list order (walrus resolves sets by index) but hide Exp/Ln
    # from every set except natural_log_exp_and_others so one load serves
    # the whole kernel.
    orig_tables = hw_specs.get_activation_tables

    def _funnel_to_combined(arch):
        t = orig_tables(arch)
        key = "natural_log_exp_and_others"
        Exp_ = mybir.ActivationFunctionType.Exp
        Ln_ = mybir.ActivationFunctionType.Ln
        out = {}
        for k, v in t.items():
            out[k] = set(v) if k == key else {f for f in v if f not in (Exp_, Ln_)}
        return out

    bacc.get_activation_tables = _funnel_to_combined

    f32 = mybir.dt.float32
    bf16 = mybir.dt.bfloat16
    fp8 = mybir.dt.float8e4
    i8 = mybir.dt.int8
    Exp = mybir.ActivationFunctionType.Exp
    Ln = mybir.ActivationFunctionType.Ln
    DR = mybir.MatmulPerfMode.DoubleRow

    nc = bacc.Bacc(
        "TRN2",
        target_bir_lowering=False,
        debug=False,
        num_devices=N_CORES,
        num_swdge_queues=4,
    )
    x = nc.dram_tensor("x", [C, T], fp8, kind="ExternalInput").ap()
    wqk = nc.dram_tensor("wqkT", [C, 2 * CG], fp8, kind="ExternalInput").ap()
    wv = nc.dram_tensor("wvT", [C, CG], fp8, kind="ExternalInput").ap()
    wp = nc.dram_tensor("wpT", [CG, C], bf16, kind="ExternalInput").ap()
    bqk = nc.dram_tensor("bqk", [2 * CG, 1], f32, kind="ExternalInput").ap()
    y = nc.dram_tensor("y", [C, T], bf16, kind="ExternalOutput").ap()

    NQ = T // 512  # 4 moving-dim chunks of 512
    NCT = T // 128  # 16 contraction chunks of 128 (s dim)
    NPR = NCT // 2  # 8 s-chunk pairs (DoubleRow k-tiles)

    with tile.TileContext(nc) as tc:
        with tc.tile_pool(name="persist", bufs=1) as pp:
            x_sb = pp.tile([128, 4, T], fp8)
            wqk_sb = pp.tile([128, 4, 2 * CG], fp8)
            wv_sb = pp.tile([128, 4, CG], fp8)
            wp_sb = pp.tile([128, 2, C], bf16)
            bqk_sb = pp.tile([128, 4], f32)
            qk_sb = pp.tile([128, 4, T], bf16)
            # v^T (+ ones) in fp8, DoubleRow layout: per s-chunk pair pr and
            # chunk parity kt, per head pair, 4 col groups of 64:
            #   [v_even | ones | ones | v_odd]
            # lhsT for the even head = groups 0:2 -> av rows [num_e; Z_e],
            # odd head = groups 2:4 -> av rows [Z_o; num_o].
            vt8 = pp.tile([128, NPR, 2, 2, 4, 64], fp8)
            attn_sb = pp.tile([128, 2, T], bf16)

            # input DMAs spread over several engine-driven queues; the k
            # projection (gate for the first scores) gets its pieces first.
            x_r = x.rearrange("(kt p) t -> p kt t", p=128)
            wqk_r = wqk.rearrange("(kt p) m -> p kt m", p=128)
            nc.gpsimd.dma_start(out=x_sb[:, 0:2, :], in_=x_r[:, 0:2, :])
            nc.scalar.dma_start(out=x_sb[:, 2:4, :], in_=x_r[:, 2:4, :])
            nc.sync.dma_start(out=wqk_sb, in_=wqk_r)
            nc.gpsimd.dma_start(
                out=wv_sb, in_=wv.rearrange("(kt p) m -> p kt m", p=128)
            )
            nc.gpsimd.dma_start(
                out=wp_sb, in_=wp.rearrange("(kt p) m -> p kt m", p=128)
            )
            nc.sync.dma_start(
                out=bqk_sb, in_=bqk.rearrange("(mt p) one -> p (mt one)", p=128)
            )
            # ones for the Z columns (v columns are overwritten below)
            nc.gpsimd.memset(vt8, 1.0)

            # ---- Phase A: q/k projections (with bias) and v^T ----
            with tc.tile_pool(name="psA", bufs=1, space="PSUM") as psA:
                # qk: out rows mt: 0 = q heads 0-1, 1 = q heads 2-3,
                #               2 = k heads 0-1, 3 = k heads 2-3
                # pair-0 tiles (mt 0, 2) first, then v^T, then pair-1 tiles:
                # the first attention block only needs mt0/mt2 + early v^T,
                # so the mt1/mt3 matmuls can overlap early attention.
                def _qk(mt, nqs):
                    pss = {}
                    for nq in nqs:
                        pss[nq] = psA.tile(
                            [128, 512], f32, tag="qk", bufs=6, name="qkps"
                        )
                    for kt in range(2):
                        for nq in nqs:
                            nc.tensor.matmul(
                                out=pss[nq],
                                lhsT=wqk_sb[
                                    :, 2 * kt : 2 * kt + 2,
                                    mt * 128 : (mt + 1) * 128,
                                ],
                                rhs=x_sb[
                                    :, 2 * kt : 2 * kt + 2,
                                    nq * 512 : (nq + 1) * 512,
                                ],
                                start=(kt == 0),
                                stop=(kt == 1),
                                perf_mode=DR,
                            )
                    # bias add on ActE: Identity(in + bias), fp32 PSUM in
                    for nq in nqs:
                        nc.scalar.activation(
                            out=qk_sb[:, mt, nq * 512 : (nq + 1) * 512],
                            in_=pss[nq],
                            func=mybir.ActivationFunctionType.Identity,
                            bias=bqk_sb[:, mt : mt + 1],
                        )

                def _vt(cts):
                    # v^T: [t, c'] tiles; no bias (folded into host-side
                    # proj bias).
                    for ct in cts:
                        ps = psA.tile([128, CG], f32, tag="vt", bufs=2)
                        for kt in range(2):
                            nc.tensor.matmul(
                                out=ps,
                                lhsT=x_sb[
                                    :, 2 * kt : 2 * kt + 2,
                                    ct * 128 : (ct + 1) * 128,
                                ],
                                rhs=wv_sb[:, 2 * kt : 2 * kt + 2, :],
                                start=(kt == 0),
                                stop=(kt == 1),
                                perf_mode=DR,
                            )
                        ps_v = ps.rearrange(
                            "p (pr parity d) -> p pr parity d", pr=2, parity=2
                        )
                        prr, kt2 = divmod(ct, 2)
                        nc.vector.tensor_copy(
                            out=vt8[:, prr, kt2, :, 0, :], in_=ps_v[:, :, 0, :]
                        )
                        nc.vector.tensor_copy(
                            out=vt8[:, prr, kt2, :, 3, :], in_=ps_v[:, :, 1, :]
                        )

                # emission order targets the first-scores gate: full k pair
                # 0, the q columns for tq=0, the first v^T chunk pair; the
                # rest streams in behind early attention.
                _qk(2, [0, 1, 2, 3])
                _qk(0, [0])
                _vt([0, 1])
                _qk(0, [1, 2, 3])
                _vt(range(2, NCT))
                _qk(3, [0, 1, 2, 3])
                _qk(1, [0, 1, 2, 3])

            # ---- Phase B: attention per head-pair p, per t-chunk tq ----
            with (
                tc.tile_pool(name="psB", bufs=1, space="PSUM") as psB,
                tc.tile_pool(name="epool", bufs=4) as epool,
                tc.tile_pool(name="rpool", bufs=3) as rpool,
            ):
                pending_norm: list = []
                for p in range(2):
                    q_t = qk_sb[:, p, :]
                    k_t = qk_sb[:, 2 + p, :]
                    for tq in range(NQ):
                        ts = slice(tq * 512, (tq + 1) * 512)
                        # fused numerator+denominator accumulators:
                        # av_e rows 0:63 = num_even, rows 64:127 = Z_even
                        # av_o rows 0:63 = Z_odd,   rows 64:127 = num_odd
                        av_e = psB.tile([128, 512], f32, tag="ave", bufs=1)
                        av_o = psB.tile([128, 512], f32, tag="avo", bufs=1)
                        for pr in range(NPR):
                            # E tile for this s-chunk pair: [head, kt, t].
                            # Per-head score tiles decouple the two exp
                            # engines: head-even -> ActE exact exp -> av_e,
                            # head-odd -> DVE Schraudolph -> av_o, so each
                            # A.V matmul waits only on its own engine.
                            e8 = epool.tile([128, 2, 2, 512], fp8, tag="e8")
                            scE = psB.tile([128, 1024], f32, tag="sc", bufs=3)
                            scO = psB.tile([128, 1024], f32, tag="sc", bufs=3)
                            for kt in range(2):
                                ct = 2 * pr + kt
                                cs = slice(ct * 128, (ct + 1) * 128)
                                ks = slice(kt * 512, (kt + 1) * 512)
                                # scores S^T[s, t] for the two heads,
                                # row-packed K=64 at partitions 0 / 64
                                # (the pair runs concurrently on the PE).
                                nc.tensor.matmul(
                                    out=scE[:, ks],
                                    lhsT=k_t[0:64, cs],
                                    rhs=q_t[0:64, ts],
                                    start=True,
                                    stop=True,
                                )
                                nc.tensor.matmul(
                                    out=scO[:, ks],
                                    lhsT=k_t[64:128, cs],
                                    rhs=q_t[64:128, ts],
                                    start=True,
                                    stop=True,
                                )
                            nc.scalar.activation(
                                out=e8[:, 0, :, :],
                                in_=scE[:],
                                func=Exp,
                                scale=SCALE,
                            )
                            nc.vector.tensor_scalar(
                                out=e8[:, 1, :, :].bitcast(i8),
                                in0=scO[:],
                                scalar1=float(SCH_A8 * SCALE),
                                scalar2=float(SCH_B8),
                                op0=mybir.AluOpType.mult,
                                op1=mybir.AluOpType.add,
                            )
                            # A.V + Z in fp8 DoubleRow: contract both chunks
                            # of the pair in one instruction per head.
                            nc.tensor.matmul(
                                out=av_e[:],
                                lhsT=vt8[:, pr, :, p, 0:2, :],
                                rhs=e8[:, 0, :, :],
                                start=(pr == 0),
                                stop=(pr == NPR - 1),
                                perf_mode=DR,
                            )
                            nc.tensor.matmul(
                                out=av_o[:],
                                lhsT=vt8[:, pr, :, p, 2:4, :],
                                rhs=e8[:, 1, :, :],
                                start=(pr == 0),
                                stop=(pr == NPR - 1),
                                perf_mode=DR,
                            )
                            # drain a deferred normalize step from the
                            # previous iteration so the recip chain never
                            # blocks this iteration's exps on ActE.
                            if pending_norm:
                                pending_norm.pop(0)()
                        # normalize: av -> SBUF bf16 (copies split across
                        # ActE/DVE so the av banks recycle fast), gather Z
                        # rows into numerator-aligned partitions (SBUF DMA),
                        # 1/Z = exp(-ln Z) (ActE), scale numerators (GPSIMD,
                        # all-SBUF operands).  Everything after the copies
                        # is deferred into the next iteration's pair loop.
                        av_sb = rpool.tile([128, 2, 512], bf16, tag="avsb")
                        nc.scalar.copy(out=av_sb[:, 0, :], in_=av_e)
                        nc.vector.tensor_copy(out=av_sb[:, 1, :], in_=av_o)
                        z2 = rpool.tile([128, 512], bf16, tag="z2")
                        nc.gpsimd.dma_start(
                            out=z2[0:64, :], in_=av_sb[64:128, 0, :]
                        )
                        nc.gpsimd.dma_start(
                            out=z2[64:128, :], in_=av_sb[0:64, 1, :]
                        )
                        lnz = rpool.tile([128, 512], f32, tag="lnz")
                        rc = rpool.tile([128, 512], bf16, tag="rc")

                        def _norm_tail(av_sb=av_sb, z2=z2, lnz=lnz, rc=rc,
                                       p=p, ts=ts):
                            return [
                                lambda: nc.scalar.activation(
                                    out=lnz, in_=z2, func=Ln
                                ),
                                lambda: nc.scalar.activation(
                                    out=rc, in_=lnz, func=Exp, scale=-1.0
                                ),
                                lambda: nc.gpsimd.tensor_mul(
                                    out=attn_sb[0:64, p, ts],
                                    in0=av_sb[0:64, 0, :],
                                    in1=rc[0:64, :],
                                ),
                                lambda: nc.gpsimd.tensor_mul(
                                    out=attn_sb[64:128, p, ts],
                                    in0=av_sb[64:128, 1, :],
                                    in1=rc[64:128, :],
                                ),
                            ]

                        pending_norm.extend(_norm_tail())
                while pending_norm:
                    pending_norm.pop(0)()

            # ---- Phase C: partial projection, t-range-major so each
            # t-range's output DMA starts as soon as it is ready ----
            with (
                tc.tile_pool(name="psC", bufs=1, space="PSUM") as psC,
                tc.tile_pool(name="ypool", bufs=2) as ypool,
            ):
                y_r = y.rearrange("(mt p) t -> p mt t", p=128)
                for nq in range(NQ):
                    for mt in range(4):
                        pj = psC.tile([128, 512], f32, tag="pj", bufs=6)
                        for kt in range(2):
                            nc.tensor.matmul(
                                out=pj,
                                lhsT=wp_sb[:, kt, mt * 128 : (mt + 1) * 128],
                                rhs=attn_sb[
                                    :, kt, nq * 512 : (nq + 1) * 512
                                ],
                                start=(kt == 0),
                                stop=(kt == 1),
                            )
                        y_t = ypool.tile([128, 512], bf16, tag="y", bufs=8)
                        if mt % 2 == 0:
                            nc.vector.tensor_copy(out=y_t, in_=pj)
                            nc.sync.dma_start(
                                out=y_r[:, mt, nq * 512 : (nq + 1) * 512],
                                in_=y_t,
                            )
                        else:
                            nc.scalar.copy(out=y_t, in_=pj)
                            nc.scalar.dma_start(
                                out=y_r[:, mt, nq * 512 : (nq + 1) * 512],
                                in_=y_t,
                            )

    nc.compile()
    return nc


def _get_runner():
    """Build (once) a cached jitted 8-core SPMD executor for the kernel."""
    if "runner" in _CACHE:
        return _CACHE["runner"]

    import jax
    import numpy as _np
    from jax.sharding import Mesh, PartitionSpec
    from jax.experimental.shard_map import shard_map

    from concourse import bass2jax, mybir

    nc = _build_nc()
    bass2jax.install_neuronx_cc_hook()

    partition_name = (
        nc.partition_id_tensor.name if nc.partition_id_tensor else None
    )
    in_names: list[str] = []
    out_names: list[str] = []
    out_avals = []
    zero_outs: list[_np.ndarray] = []
    for alloc in nc.m.functions[0].allocations:
        if not isinstance(alloc, mybir.MemoryLocationSet):
            continue
        name = alloc.memorylocations[0].name
        if alloc.kind == "ExternalInput":
            if name != partition_name:
                in_names.append(name)
        elif alloc.kind == "ExternalOutput":
            shape = tuple(alloc.tensor_shape)
            dtype = mybir.dt.np(alloc.dtype)
            out_names.append(name)
            out_avals.append(jax.core.ShapedArray(shape, dtype))
            zero_outs.append(_np.zeros(shape, dtype))
    n_params = len(in_names)
    n_outs = len(out_avals)
    in_names_all = in_names + out_names
    if partition_name is not None:
        in_names_all.append(partition_name)

    donate = tuple(range(n_params, n_params + n_outs))

    def _body(*args):
        operands = list(args)
        if partition_name is not None:
            operands.append(bass2jax.partition_id_tensor())
        outs = bass2jax._bass_exec_p.bind(
            *operands,
            out_avals=tuple(out_avals),
            in_names=tuple(in_names_all),
            out_names=tuple(out_names),
            lowering_input_output_aliases=(),
            sim_require_finite=True,
            sim_require_nnan=True,
            nc=nc,
        )
        return tuple(outs)

    devices = jax.devices()[:N_CORES]
    mesh = Mesh(np.asarray(devices), ("core",))
    in_specs = (PartitionSpec("core"),) * (n_params + n_outs)
    out_specs = (PartitionSpec("core"),) * n_outs
    sharded = jax.jit(
        shard_map(
            _body, mesh=mesh, in_specs=in_specs, out_specs=out_specs,
            check_rep=False,
        ),
        donate_argnums=donate,
        keep_unused=True,
    )

    runner = {
        "fn": sharded,
        "in_names": in_names,
        "out_names": out_names,
        "zero_outs": zero_outs,
    }
    _CACHE["runner"] = runner
    return runner


def _prepare_in_maps(x, qkv_w, qkv_b, proj_w, proj_b):
    """Full inputs -> per-core input dicts (batch x head-group sharding)."""
    in_maps = []
    for c in range(N_CORES):
        b, g = divmod(c, G)
        qs = slice(g * CG, (g + 1) * CG)
        wq = qkv_w[qs, :]
        wk = qkv_w[C + g * CG : C + (g + 1) * CG, :]
        wv = qkv_w[2 * C + g * CG : 2 * C + (g + 1) * CG, :]
        wqkT = np.ascontiguousarray(np.concatenate([wq, wk], axis=0).T)
        wvT = np.ascontiguousarray(wv.T)
        wpT = np.ascontiguousarray(proj_w[:, qs].T)
        bqk = np.ascontiguousarray(
            np.concatenate([qkv_b[qs], qkv_b[C + g * CG : C + (g + 1) * CG]])
        ).reshape(2 * CG, 1)
        in_maps.append(
            {
                "x": np.ascontiguousarray(x[b]).astype(_FP8),
                "wqkT": wqkT.astype(_FP8),
                "wvT": wvT.astype(_FP8),
                "wpT": wpT.astype(_BF16),
                "bqk": bqk.astype(np.float32),
            }
        )
    return in_maps


def _run_in_maps(in_maps):
    """Run the SPMD kernel, return list of per-core output dicts."""
    r = _get_runner()
    per_core = [
        [np.asarray(m[name]) for name in r["in_names"]] for m in in_maps
    ]
    concat_in = [
        np.concatenate([per_core[c][i] for c in range(N_CORES)], axis=0)
        for i in range(len(r["in_names"]))
    ]
    concat_zero = [
        np.concatenate([z] * N_CORES, axis=0) for z in r["zero_outs"]
    ]
    outs = r["fn"](*concat_in, *concat_zero)
    outs = [np.asarray(o) for o in outs]
    results = []
    for c in range(N_CORES):
        d = {}
        for i, name in enumerate(r["out_names"]):
            per_len = outs[i].shape[0] // N_CORES
            d[name] = outs[i][c * per_len : (c + 1) * per_len]
        results.append(d)
    return results


def kernel(x, qkv_w, qkv_b, proj_w, proj_b):
    x = np.asarray(x, dtype=np.float32)
    qkv_w = np.asarray(qkv_w, dtype=np.float32)
    qkv_b = np.asarray(qkv_b, dtype=np.float32)
    proj_w = np.asarray(proj_w, dtype=np.float32)
    proj_b = np.asarray(proj_b, dtype=np.float32)

    in_maps = _prepare_in_maps(x, qkv_w, qkv_b, proj_w, proj_b)
    results = _run_in_maps(in_maps)

    # host combine: residual + effective projection bias + the two
    # head-group partials per batch.
    bp_eff = proj_b + proj_w @ qkv_b[2 * C : 3 * C]
    out = np.empty((B, C, T), dtype=np.float32)
    for b in range(B):
        out[b] = (
            x[b]
            + bp_eff[:, None]
            + results[G * b]["y"]
            + results[G * b + 1]["y"]
        )
    return out


# revision 27
# speedup vs baseline: 1.5981x; 1.5981x over previous
"""Attention1D Trainium2 Bass kernel, sharded over 8 NeuronCores.

Reference computation (per batch b, C=512 channels, T=2048, H=8 heads, d=64):
    qkv = qkv_w @ x + qkv_b            # [3C, T]
    q, k, v = split(qkv)               # each [C, T], viewed as H heads of d=64
    attn = softmax((q_h . k_h) * C**-0.5, over s)
    out_h = attn @ v_h
    out = proj_w @ concat(out_h) + proj_b
    result = x + out

Sharding: 8 cores = 4 batches x 2 head-groups (4 heads each).  Each core
computes its group's partial projection output y_g = proj_w[:, g] @ attn_g;
the host combines: out[b] = x[b] + (proj_b + proj_w @ v_bias) + y_0 + y_1.
(The v bias commutes through softmax-weighted averaging because the
probabilities sum to 1, so it folds into an effective projection bias.)

v2 design (bottleneck: ActE exp at ~156us busy in v1, PE ~154us):
  * Because the softmax scale is C**-0.5 (full channel count), logits are
    tiny (~N(0, 0.35)), so E = exp(logit) is in [~0.1, ~8] - comfortably
    inside fp8e4m3 range.  E and v^T are stored fp8 and the A.V matmul runs
    in fp8 DoubleRow mode (2 s-chunks contracted per instruction at 0.5
    cycles/row), halving its PE cost vs bf16.
  * The exp work is split between ActE (exact Exp activation, fp8 out) and
    the otherwise-idle Vector engine, which computes exp with a Schraudolph
    bit-trick: bits8 = rint(logit * 8/ln2 + (56 - 0.37)) interpreted as
    fp8e4m3 (one fused tensor_scalar mult+add with int8 output; hw rounds
    rint, verified).  ~3% rms error on attention weights, which cancels in
    the softmax normalization and averages out in A.V.
  * Softmax denominators: ones-columns packed into the DR stationary give
    Z rows inside the A.V PSUM accumulators for free.  Normalize path:
    ActE copies av PSUM -> SBUF bf16, SBUF->SBUF DMA gathers Z rows into
    numerator-aligned partitions, ActE exp(-ln Z) gives reciprocals, DVE
    bf16 2x-mode muls write the normalized attention output.
"""

import sys

if "/opt/trn_rl_repo" not in sys.path:
    sys.path.insert(0, "/opt/trn_rl_repo")

import numpy as np

try:
    import ml_dtypes

    _BF16 = ml_dtypes.bfloat16
    _FP8 = ml_dtypes.float8_e4m3
except ImportError:  # pragma: no cover
    _BF16 = None
    _FP8 = None

B, C, T, H = 4, 512, 2048, 8
D = C // H  # 64 head dim
G = 2  # head groups (cores per batch)
CG = C // G  # 256 channels per group
SCALE = float(C) ** -0.5
N_CORES = 8

# Schraudolph exp-as-int8 constants (fp8e4m3 bits; hw rint rounding).
SCH_A8 = 8.0 / float(np.log(2.0))
SCH_B8 = 7.0 * 8.0 - 0.37
# Fraction of the 128 exp tiles handled by ActE (exact); rest on DVE.
ACT_EXP_TILES = 62

_CACHE: dict = {}


def _build_nc():
    import concourse.tile as tile
    from concourse import bacc, mybir
    from concourse import hw_specs

    # Our only ScalarE table functions are Exp and Ln.  The table-load pass
    # picks the first set (by canonical act_info.json index) containing each
    # function, which would alternate between the exp-only set and the
    # ln+exp set and reload tables at every normalization.  Keep the
    # canonical list order (walrus resolves sets by index) but hide Exp/Ln
    # from every set except natural_log_exp_and_others so one load serves
    # the whole kernel.
    orig_tables = hw_specs.get_activation_tables

    def _funnel_to_combined(arch):
        t = orig_tables(arch)
        key = "natural_log_exp_and_others"
        Exp_ = mybir.ActivationFunctionType.Exp
        Ln_ = mybir.ActivationFunctionType.Ln
        out = {}
        for k, v in t.items():
            out[k] = set(v) if k == key else {f for f in v if f not in (Exp_, Ln_)}
        return out

    bacc.get_activation_tables = _funnel_to_combined

    f32 = mybir.dt.float32
    bf16 = mybir.dt.bfloat16
    fp8 = mybir.dt.float8e4
    i8 = mybir.dt.int8
    Exp = mybir.ActivationFunctionType.Exp
    Ln = mybir.ActivationFunctionType.Ln
    DR = mybir.MatmulPerfMode.DoubleRow

    nc = bacc.Bacc(
        "TRN2",
        target_bir_lowering=False,
        debug=False,
        num_devices=N_CORES,
        num_swdge_queues=4,
    )
    x = nc.dram_tensor("x", [C, T], fp8, kind="ExternalInput").ap()
    wqk = nc.dram_tensor("wqkT", [C, 2 * CG], fp8, kind="ExternalInput").ap()
    wv = nc.dram_tensor("wvT", [C, CG], fp8, kind="ExternalInput").ap()
    wp = nc.dram_tensor("wpT", [CG, C], bf16, kind="ExternalInput").ap()
    bqk = nc.dram_tensor("bqk", [2 * CG, 1], f32, kind="ExternalInput").ap()
    y = nc.dram_tensor("y", [C, T], bf16, kind="ExternalOutput").ap()

    NQ = T // 512  # 4 moving-dim chunks of 512
    NCT = T // 128  # 16 contraction chunks of 128 (s dim)
    NPR = NCT // 2  # 8 s-chunk pairs (DoubleRow k-tiles)

    with tile.TileContext(nc) as tc:
        with tc.tile_pool(name="persist", bufs=1) as pp:
            x_sb = pp.tile([128, 4, T], fp8)
            wqk_sb = pp.tile([128, 4, 2 * CG], fp8)
            wv_sb = pp.tile([128, 4, CG], fp8)
            wp_sb = pp.tile([128, 2, C], bf16)
            bqk_sb = pp.tile([128, 4], f32)
            qk_sb = pp.tile([128, 4, T], bf16)
            # v^T (+ ones) in fp8, DoubleRow layout: per s-chunk pair pr and
            # chunk parity kt, per head pair, 4 col groups of 64:
            #   [v_even | ones | ones | v_odd]
            # lhsT for the even head = groups 0:2 -> av rows [num_e; Z_e],
            # odd head = groups 2:4 -> av rows [Z_o; num_o].
            vt8 = pp.tile([128, NPR, 2, 2, 4, 64], fp8)
            attn_sb = pp.tile([128, 2, T], bf16)

            # input DMAs spread over several engine-driven queues; the k
            # projection (gate for the first scores) gets its pieces first.
            x_r = x.rearrange("(kt p) t -> p kt t", p=128)
            wqk_r = wqk.rearrange("(kt p) m -> p kt m", p=128)
            nc.gpsimd.dma_start(out=x_sb[:, 0:2, :], in_=x_r[:, 0:2, :])
            nc.scalar.dma_start(out=x_sb[:, 2:4, :], in_=x_r[:, 2:4, :])
            nc.sync.dma_start(out=wqk_sb, in_=wqk_r)
            nc.gpsimd.dma_start(
                out=wv_sb, in_=wv.rearrange("(kt p) m -> p kt m", p=128)
            )
            nc.gpsimd.dma_start(
                out=wp_sb, in_=wp.rearrange("(kt p) m -> p kt m", p=128)
            )
            nc.sync.dma_start(
                out=bqk_sb, in_=bqk.rearrange("(mt p) one -> p (mt one)", p=128)
            )
            # ones for the Z columns (v columns are overwritten below)
            nc.gpsimd.memset(vt8, 1.0)

            # ---- Phase A: q/k projections (with bias) and v^T ----
            with tc.tile_pool(name="psA", bufs=1, space="PSUM") as psA:
                # qk: out rows mt: 0 = q heads 0-1, 1 = q heads 2-3,
                #               2 = k heads 0-1, 3 = k heads 2-3
                # pair-0 tiles (mt 0, 2) first, then v^T, then pair-1 tiles:
                # the first attention block only needs mt0/mt2 + early v^T,
                # so the mt1/mt3 matmuls can overlap early attention.
                def _qk(mt, nqs):
                    pss = {}
                    for nq in nqs:
                        pss[nq] = psA.tile(
                            [128, 512], f32, tag="qk", bufs=6, name="qkps"
                        )
                    for kt in range(2):
                        for nq in nqs:
                            nc.tensor.matmul(
                                out=pss[nq],
                                lhsT=wqk_sb[
                                    :, 2 * kt : 2 * kt + 2,
                                    mt * 128 : (mt + 1) * 128,
                                ],
                                rhs=x_sb[
                                    :, 2 * kt : 2 * kt + 2,
                                    nq * 512 : (nq + 1) * 512,
                                ],
                                start=(kt == 0),
                                stop=(kt == 1),
                                perf_mode=DR,
                            )
                    # bias add on ActE: Identity(in + bias), fp32 PSUM in
                    for nq in nqs:
                        nc.scalar.activation(
                            out=qk_sb[:, mt, nq * 512 : (nq + 1) * 512],
                            in_=pss[nq],
                            func=mybir.ActivationFunctionType.Identity,
                            bias=bqk_sb[:, mt : mt + 1],
                        )

                def _vt(cts):
                    # v^T: [t, c'] tiles; no bias (folded into host-side
                    # proj bias).
                    for ct in cts:
                        ps = psA.tile([128, CG], f32, tag="vt", bufs=2)
                        for kt in range(2):
                            nc.tensor.matmul(
                                out=ps,
                                lhsT=x_sb[
                                    :, 2 * kt : 2 * kt + 2,
                                    ct * 128 : (ct + 1) * 128,
                                ],
                                rhs=wv_sb[:, 2 * kt : 2 * kt + 2, :],
                                start=(kt == 0),
                                stop=(kt == 1),
                                perf_mode=DR,
                            )
                        ps_v = ps.rearrange(
                            "p (pr parity d) -> p pr parity d", pr=2, parity=2
                        )
                        prr, kt2 = divmod(ct, 2)
                        nc.vector.tensor_copy(
                            out=vt8[:, prr, kt2, :, 0, :], in_=ps_v[:, :, 0, :]
                        )
                        nc.vector.tensor_copy(
                            out=vt8[:, prr, kt2, :, 3, :], in_=ps_v[:, :, 1, :]
                        )

                # emission order targets the first-scores gate: full k pair
                # 0, the q columns for tq=0, the first v^T chunk pair; the
                # rest streams in behind early attention.
                _qk(2, [0, 1, 2, 3])
                _qk(0, [0])
                _vt([0, 1])
                _qk(0, [1, 2, 3])
                _vt(range(2, NCT))
                _qk(3, [0, 1, 2, 3])
                _qk(1, [0, 1, 2, 3])

            # ---- Phase B: attention per head-pair p, per t-chunk tq ----
            with (
                tc.tile_pool(name="psB", bufs=1, space="PSUM") as psB,
                tc.tile_pool(name="epool", bufs=4) as epool,
                tc.tile_pool(name="rpool", bufs=3) as rpool,
            ):
                pending_norm: list = []
                for p in range(2):
                    q_t = qk_sb[:, p, :]
                    k_t = qk_sb[:, 2 + p, :]
                    for tq in range(NQ):
                        ts = slice(tq * 512, (tq + 1) * 512)
                        # fused numerator+denominator accumulators:
                        # av_e rows 0:63 = num_even, rows 64:127 = Z_even
                        # av_o rows 0:63 = Z_odd,   rows 64:127 = num_odd
                        av_e = psB.tile([128, 512], f32, tag="ave", bufs=1)
                        av_o = psB.tile([128, 512], f32, tag="avo", bufs=1)
                        def _av(pr, e8):
                            # A.V + Z in fp8 DoubleRow: contract both chunks
                            # of the pair in one instruction per head.
                            nc.tensor.matmul(
                                out=av_e[:],
                                lhsT=vt8[:, pr, :, p, 0:2, :],
                                rhs=e8[:, 0, :, :],
                                start=(pr == 0),
                                stop=(pr == NPR - 1),
                                perf_mode=DR,
                            )
                            nc.tensor.matmul(
                                out=av_o[:],
                                lhsT=vt8[:, pr, :, p, 2:4, :],
                                rhs=e8[:, 1, :, :],
                                start=(pr == 0),
                                stop=(pr == NPR - 1),
                                perf_mode=DR,
                            )

                        prev_av = None
                        for pr in range(NPR):
                            # E tile for this s-chunk pair: [head, kt, t]
                            e8 = epool.tile([128, 2, 2, 512], fp8, tag="e8")
                            for kt in range(2):
                                ct = 2 * pr + kt
                                cs = slice(ct * 128, (ct + 1) * 128)
                                sc = psB.tile(
                                    [128, 1024], f32, tag="sc", bufs=3
                                )
                                # scores S^T[s, t] for the two heads,
                                # row-packed K=64 at partitions 0 / 64.
                                nc.tensor.matmul(
                                    out=sc[:, 0:512],
                                    lhsT=k_t[0:64, cs],
                                    rhs=q_t[0:64, ts],
                                    start=True,
                                    stop=True,
                                )
                                nc.tensor.matmul(
                                    out=sc[:, 512:1024],
                                    lhsT=k_t[64:128, cs],
                                    rhs=q_t[64:128, ts],
                                    start=True,
                                    stop=True,
                                )
                                # DVE (slower Schraudolph) gets the earlier
                                # chunk, ActE exact exp the later one.
                                e_out = e8[:, :, kt, :]
                                if kt == 0:
                                    nc.vector.tensor_scalar(
                                        out=e_out.bitcast(i8),
                                        in0=sc[:],
                                        scalar1=float(SCH_A8 * SCALE),
                                        scalar2=float(SCH_B8),
                                        op0=mybir.AluOpType.mult,
                                        op1=mybir.AluOpType.add,
                                    )
                                else:
                                    nc.scalar.activation(
                                        out=e_out,
                                        in_=sc[:],
                                        func=Exp,
                                        scale=SCALE,
                                    )
                            # software-pipeline: issue the PREVIOUS pair's
                            # A.V matmuls now, so the tensor engine never
                            # waits on an in-flight exp.
                            if prev_av is not None:
                                _av(*prev_av)
                            prev_av = (pr, e8)
                            # drain a deferred normalize step from the
                            # previous iteration so the recip chain never
                            # blocks this iteration's exps on ActE.
                            if pending_norm:
                                pending_norm.pop(0)()
                        _av(*prev_av)
                        # normalize: av -> SBUF bf16 (copies split across
                        # ActE/DVE so the av banks recycle fast), gather Z
                        # rows into numerator-aligned partitions (SBUF DMA),
                        # 1/Z = exp(-ln Z) (ActE), scale numerators (GPSIMD,
                        # all-SBUF operands).  Everything after the copies
                        # is deferred into the next iteration's pair loop.
                        av_sb = rpool.tile([128, 2, 512], bf16, tag="avsb")
                        nc.scalar.copy(out=av_sb[:, 0, :], in_=av_e)
                        nc.vector.tensor_copy(out=av_sb[:, 1, :], in_=av_o)
                        z2 = rpool.tile([128, 512], bf16, tag="z2")
                        nc.gpsimd.dma_start(
                            out=z2[0:64, :], in_=av_sb[64:128, 0, :]
                        )
                        nc.gpsimd.dma_start(
                            out=z2[64:128, :], in_=av_sb[0:64, 1, :]
                        )
                        lnz = rpool.tile([128, 512], f32, tag="lnz")
                        rc = rpool.tile([128, 512], bf16, tag="rc")

                        def _norm_tail(av_sb=av_sb, z2=z2, lnz=lnz, rc=rc,
                                       p=p, ts=ts):
                            return [
                                lambda: nc.scalar.activation(
                                    out=lnz, in_=z2, func=Ln
                                ),
                                lambda: nc.scalar.activation(
                                    out=rc, in_=lnz, func=Exp, scale=-1.0
                                ),
                                lambda: nc.gpsimd.tensor_mul(
                                    out=attn_sb[0:64, p, ts],
                                    in0=av_sb[0:64, 0, :],
                                    in1=rc[0:64, :],
                                ),
                                lambda: nc.gpsimd.tensor_mul(
                                    out=attn_sb[64:128, p, ts],
                                    in0=av_sb[64:128, 1, :],
                                    in1=rc[64:128, :],
                                ),
                            ]

                        pending_norm.extend(_norm_tail())
                while pending_norm:
                    pending_norm.pop(0)()

            # ---- Phase C: partial projection, t-range-major so each
            # t-range's output DMA starts as soon as it is ready ----
            with (
                tc.tile_pool(name="psC", bufs=1, space="PSUM") as psC,
                tc.tile_pool(name="ypool", bufs=2) as ypool,
            ):
                y_r = y.rearrange("(mt p) t -> p mt t", p=128)
                for nq in range(NQ):
                    for mt in range(4):
                        pj = psC.tile([128, 512], f32, tag="pj", bufs=6)
                        for kt in range(2):
                            nc.tensor.matmul(
                                out=pj,
                                lhsT=wp_sb[:, kt, mt * 128 : (mt + 1) * 128],
                                rhs=attn_sb[
                                    :, kt, nq * 512 : (nq + 1) * 512
                                ],
                                start=(kt == 0),
                                stop=(kt == 1),
                            )
                        y_t = ypool.tile([128, 512], bf16, tag="y", bufs=8)
                        if mt % 2 == 0:
                            nc.vector.tensor_copy(out=y_t, in_=pj)
                            nc.sync.dma_start(
                                out=y_r[:, mt, nq * 512 : (nq + 1) * 512],
                                in_=y_t,
                            )
                        else:
                            nc.scalar.copy(out=y_t, in_=pj)
                            nc.scalar.dma_start(
                                out=y_r[:, mt, nq * 512 : (nq + 1) * 512],
                                in_=y_t,
                            )

    nc.compile()
    return nc


def _get_runner():
    """Build (once) a cached jitted 8-core SPMD executor for the kernel."""
    if "runner" in _CACHE:
        return _CACHE["runner"]

    import jax
    import numpy as _np
    from jax.sharding import Mesh, PartitionSpec
    from jax.experimental.shard_map import shard_map

    from concourse import bass2jax, mybir

    nc = _build_nc()
    bass2jax.install_neuronx_cc_hook()

    partition_name = (
        nc.partition_id_tensor.name if nc.partition_id_tensor else None
    )
    in_names: list[str] = []
    out_names: list[str] = []
    out_avals = []
    zero_outs: list[_np.ndarray] = []
    for alloc in nc.m.functions[0].allocations:
        if not isinstance(alloc, mybir.MemoryLocationSet):
            continue
        name = alloc.memorylocations[0].name
        if alloc.kind == "ExternalInput":
            if name != partition_name:
                in_names.append(name)
        elif alloc.kind == "ExternalOutput":
            shape = tuple(alloc.tensor_shape)
            dtype = mybir.dt.np(alloc.dtype)
            out_names.append(name)
            out_avals.append(jax.core.ShapedArray(shape, dtype))
            zero_outs.append(_np.zeros(shape, dtype))
    n_params = len(in_names)
    n_outs = len(out_avals)
    in_names_all = in_names + out_names
    if partition_name is not None:
        in_names_all.append(partition_name)

    donate = tuple(range(n_params, n_params + n_outs))

    def _body(*args):
        operands = list(args)
        if partition_name is not None:
            operands.append(bass2jax.partition_id_tensor())
        outs = bass2jax._bass_exec_p.bind(
            *operands,
            out_avals=tuple(out_avals),
            in_names=tuple(in_names_all),
            out_names=tuple(out_names),
            lowering_input_output_aliases=(),
            sim_require_finite=True,
            sim_require_nnan=True,
            nc=nc,
        )
        return tuple(outs)

    devices = jax.devices()[:N_CORES]
    mesh = Mesh(np.asarray(devices), ("core",))
    in_specs = (PartitionSpec("core"),) * (n_params + n_outs)
    out_specs = (PartitionSpec("core"),) * n_outs
    sharded = jax.jit(
        shard_map(
            _body, mesh=mesh, in_specs=in_specs, out_specs=out_specs,
            check_rep=False,
        ),
        donate_argnums=donate,
        keep_unused=True,
    )

    runner = {
        "fn": sharded,
        "in_names": in_names,
        "out_names": out_names,
        "zero_outs": zero_outs,
    }
    _CACHE["runner"] = runner
    return runner


def _prepare_in_maps(x, qkv_w, qkv_b, proj_w, proj_b):
    """Full inputs -> per-core input dicts (batch x head-group sharding)."""
    in_maps = []
    for c in range(N_CORES):
        b, g = divmod(c, G)
        qs = slice(g * CG, (g + 1) * CG)
        wq = qkv_w[qs, :]
        wk = qkv_w[C + g * CG : C + (g + 1) * CG, :]
        wv = qkv_w[2 * C + g * CG : 2 * C + (g + 1) * CG, :]
        wqkT = np.ascontiguousarray(np.concatenate([wq, wk], axis=0).T)
        wvT = np.ascontiguousarray(wv.T)
        wpT = np.ascontiguousarray(proj_w[:, qs].T)
        bqk = np.ascontiguousarray(
            np.concatenate([qkv_b[qs], qkv_b[C + g * CG : C + (g + 1) * CG]])
        ).reshape(2 * CG, 1)
        in_maps.append(
            {
                "x": np.ascontiguousarray(x[b]).astype(_FP8),
                "wqkT": wqkT.astype(_FP8),
                "wvT": wvT.astype(_FP8),
                "wpT": wpT.astype(_BF16),
                "bqk": bqk.astype(np.float32),
            }
        )
    return in_maps


def _run_in_maps(in_maps):
    """Run the SPMD kernel, return list of per-core output dicts."""
    r = _get_runner()
    per_core = [
        [np.asarray(m[name]) for name in r["in_names"]] for m in in_maps
    ]
    concat_in = [
        np.concatenate([per_core[c][i] for c in range(N_CORES)], axis=0)
        for i in range(len(r["in_names"]))
    ]
    concat_zero = [
        np.concatenate([z] * N_CORES, axis=0) for z in r["zero_outs"]
    ]
    outs = r["fn"](*concat_in, *concat_zero)
    outs = [np.asarray(o) for o in outs]
    results = []
    for c in range(N_CORES):
        d = {}
        for i, name in enumerate(r["out_names"]):
            per_len = outs[i].shape[0] // N_CORES
            d[name] = outs[i][c * per_len : (c + 1) * per_len]
        results.append(d)
    return results


def kernel(x, qkv_w, qkv_b, proj_w, proj_b):
    x = np.asarray(x, dtype=np.float32)
    qkv_w = np.asarray(qkv_w, dtype=np.float32)
    qkv_b = np.asarray(qkv_b, dtype=np.float32)
    proj_w = np.asarray(proj_w, dtype=np.float32)
    proj_b = np.asarray(proj_b, dtype=np.float32)

    in_maps = _prepare_in_maps(x, qkv_w, qkv_b, proj_w, proj_b)
    results = _run_in_maps(in_maps)

    # host combine: residual + effective projection bias + the two
    # head-group partials per batch.
    bp_eff = proj_b + proj_w @ qkv_b[2 * C : 3 * C]
    out = np.empty((B, C, T), dtype=np.float32)
    for b in range(B):
        out[b] = (
            x[b]
            + bp_eff[:, None]
            + results[G * b]["y"]
            + results[G * b + 1]["y"]
        )
    return out


# revision 31
# speedup vs baseline: 1.6322x; 1.0213x over previous
"""Attention1D Trainium2 Bass kernel, sharded over 8 NeuronCores.

Reference computation (per batch b, C=512 channels, T=2048, H=8 heads, d=64):
    qkv = qkv_w @ x + qkv_b            # [3C, T]
    q, k, v = split(qkv)               # each [C, T], viewed as H heads of d=64
    attn = softmax((q_h . k_h) * C**-0.5, over s)
    out_h = attn @ v_h
    out = proj_w @ concat(out_h) + proj_b
    result = x + out

Sharding: 8 cores = 4 batches x 2 head-groups (4 heads each).  Each core
computes its group's partial projection output y_g = proj_w[:, g] @ attn_g;
the host combines: out[b] = x[b] + (proj_b + proj_w @ v_bias) + y_0 + y_1.
(The v bias commutes through softmax-weighted averaging because the
probabilities sum to 1, so it folds into an effective projection bias.)

v2 design (bottleneck: ActE exp at ~156us busy in v1, PE ~154us):
  * Because the softmax scale is C**-0.5 (full channel count), logits are
    tiny (~N(0, 0.35)), so E = exp(logit) is in [~0.1, ~8] - comfortably
    inside fp8e4m3 range.  E and v^T are stored fp8 and the A.V matmul runs
    in fp8 DoubleRow mode (2 s-chunks contracted per instruction at 0.5
    cycles/row), halving its PE cost vs bf16.
  * The exp work is split between ActE (exact Exp activation, fp8 out) and
    the otherwise-idle Vector engine, which computes exp with a Schraudolph
    bit-trick: bits8 = rint(logit * 8/ln2 + (56 - 0.37)) interpreted as
    fp8e4m3 (one fused tensor_scalar mult+add with int8 output; hw rounds
    rint, verified).  ~3% rms error on attention weights, which cancels in
    the softmax normalization and averages out in A.V.
  * Softmax denominators: ones-columns packed into the DR stationary give
    Z rows inside the A.V PSUM accumulators for free.  Normalize path:
    ActE copies av PSUM -> SBUF bf16, SBUF->SBUF DMA gathers Z rows into
    numerator-aligned partitions, ActE exp(-ln Z) gives reciprocals, DVE
    bf16 2x-mode muls write the normalized attention output.
"""

import sys

if "/opt/trn_rl_repo" not in sys.path:
    sys.path.insert(0, "/opt/trn_rl_repo")

import numpy as np

try:
    import ml_dtypes

    _BF16 = ml_dtypes.bfloat16
    _FP8 = ml_dtypes.float8_e4m3
except ImportError:  # pragma: no cover
    _BF16 = None
    _FP8 = None

B, C, T, H = 4, 512, 2048, 8
D = C // H  # 64 head dim
G = 2  # head groups (cores per batch)
CG = C // G  # 256 channels per group
SCALE = float(C) ** -0.5
N_CORES = 8

# Schraudolph exp-as-int8 constants (fp8e4m3 bits; hw rint rounding).
SCH_A8 = 8.0 / float(np.log(2.0))
SCH_B8 = 7.0 * 8.0 - 0.37
# Fraction of the 128 exp tiles handled by ActE (exact); rest on DVE.
ACT_EXP_TILES = 62

_CACHE: dict = {}


def _build_nc():
    import concourse.tile as tile
    from concourse import bacc, mybir
    from concourse import hw_specs

    # Our only ScalarE table functions are Exp and Ln.  The table-load pass
    # picks the first set (by canonical act_info.json index) containing each
    # function, which would alternate between the exp-only set and the
    # ln+exp set and reload tables at every normalization.  Keep the
    # canonical list order (walrus resolves sets by index) but hide Exp/Ln
    # from every set except natural_log_exp_and_others so one load serves
    # the whole kernel.
    orig_tables = hw_specs.get_activation_tables

    def _funnel_to_combined(arch):
        t = orig_tables(arch)
        key = "natural_log_exp_and_others"
        Exp_ = mybir.ActivationFunctionType.Exp
        Ln_ = mybir.ActivationFunctionType.Ln
        out = {}
        for k, v in t.items():
            out[k] = set(v) if k == key else {f for f in v if f not in (Exp_, Ln_)}
        return out

    bacc.get_activation_tables = _funnel_to_combined

    f32 = mybir.dt.float32
    bf16 = mybir.dt.bfloat16
    fp8 = mybir.dt.float8e4
    i8 = mybir.dt.int8
    Exp = mybir.ActivationFunctionType.Exp
    Ln = mybir.ActivationFunctionType.Ln
    DR = mybir.MatmulPerfMode.DoubleRow

    nc = bacc.Bacc(
        "TRN2",
        target_bir_lowering=False,
        debug=False,
        num_devices=N_CORES,
        num_swdge_queues=4,
    )
    x = nc.dram_tensor("x", [C, T], fp8, kind="ExternalInput").ap()
    wqk = nc.dram_tensor("wqkT", [C, 2 * CG], fp8, kind="ExternalInput").ap()
    wv = nc.dram_tensor("wvT", [C, CG], fp8, kind="ExternalInput").ap()
    wp = nc.dram_tensor("wpT", [CG, C], bf16, kind="ExternalInput").ap()
    bqk = nc.dram_tensor("bqk", [2 * CG, 1], f32, kind="ExternalInput").ap()
    y = nc.dram_tensor("y", [C, T], bf16, kind="ExternalOutput").ap()

    NQ = T // 512  # 4 moving-dim chunks of 512
    NCT = T // 128  # 16 contraction chunks of 128 (s dim)
    NPR = NCT // 2  # 8 s-chunk pairs (DoubleRow k-tiles)

    with tile.TileContext(nc) as tc:
        with tc.tile_pool(name="persist", bufs=1) as pp:
            x_sb = pp.tile([128, 4, T], fp8)
            wqk_sb = pp.tile([128, 4, 2 * CG], fp8)
            wv_sb = pp.tile([128, 4, CG], fp8)
            wp_sb = pp.tile([128, 2, C], bf16)
            bqk_sb = pp.tile([128, 4], f32)
            qk_sb = pp.tile([128, 4, T], bf16)
            # v^T (+ ones) in fp8, DoubleRow layout: per s-chunk pair pr and
            # chunk parity kt, per head pair, 4 col groups of 64:
            #   [v_even | ones | ones | v_odd]
            # lhsT for the even head = groups 0:2 -> av rows [num_e; Z_e],
            # odd head = groups 2:4 -> av rows [Z_o; num_o].
            vt8 = pp.tile([128, NPR, 2, 2, 4, 64], fp8)
            attn_sb = pp.tile([128, 2, T], bf16)

            # input DMAs spread over several engine-driven queues; the k
            # projection (gate for the first scores) gets its pieces first.
            x_r = x.rearrange("(kt p) t -> p kt t", p=128)
            wqk_r = wqk.rearrange("(kt p) m -> p kt m", p=128)
            nc.gpsimd.dma_start(out=x_sb[:, 0:2, :], in_=x_r[:, 0:2, :])
            nc.scalar.dma_start(out=x_sb[:, 2:4, :], in_=x_r[:, 2:4, :])
            nc.sync.dma_start(out=wqk_sb, in_=wqk_r)
            nc.gpsimd.dma_start(
                out=wv_sb, in_=wv.rearrange("(kt p) m -> p kt m", p=128)
            )
            nc.gpsimd.dma_start(
                out=wp_sb, in_=wp.rearrange("(kt p) m -> p kt m", p=128)
            )
            nc.sync.dma_start(
                out=bqk_sb, in_=bqk.rearrange("(mt p) one -> p (mt one)", p=128)
            )
            # ones for the Z columns (v columns are overwritten below)
            nc.gpsimd.memset(vt8, 1.0)

            # ---- Phase A: q/k projections (with bias) and v^T ----
            with tc.tile_pool(name="psA", bufs=1, space="PSUM") as psA:
                # qk: out rows mt: 0 = q heads 0-1, 1 = q heads 2-3,
                #               2 = k heads 0-1, 3 = k heads 2-3
                # pair-0 tiles (mt 0, 2) first, then v^T, then pair-1 tiles:
                # the first attention block only needs mt0/mt2 + early v^T,
                # so the mt1/mt3 matmuls can overlap early attention.
                def _qk(mt, nqs):
                    pss = {}
                    for nq in nqs:
                        pss[nq] = psA.tile(
                            [128, 512], f32, tag="qk", bufs=6, name="qkps"
                        )
                    for kt in range(2):
                        for nq in nqs:
                            nc.tensor.matmul(
                                out=pss[nq],
                                lhsT=wqk_sb[
                                    :, 2 * kt : 2 * kt + 2,
                                    mt * 128 : (mt + 1) * 128,
                                ],
                                rhs=x_sb[
                                    :, 2 * kt : 2 * kt + 2,
                                    nq * 512 : (nq + 1) * 512,
                                ],
                                start=(kt == 0),
                                stop=(kt == 1),
                                perf_mode=DR,
                            )
                    # bias add on ActE: Identity(in + bias), fp32 PSUM in
                    for nq in nqs:
                        nc.scalar.activation(
                            out=qk_sb[:, mt, nq * 512 : (nq + 1) * 512],
                            in_=pss[nq],
                            func=mybir.ActivationFunctionType.Identity,
                            bias=bqk_sb[:, mt : mt + 1],
                        )

                def _vt(cts):
                    # v^T: [t, c'] tiles; no bias (folded into host-side
                    # proj bias).
                    for ct in cts:
                        ps = psA.tile([128, CG], f32, tag="vt", bufs=2)
                        for kt in range(2):
                            nc.tensor.matmul(
                                out=ps,
                                lhsT=x_sb[
                                    :, 2 * kt : 2 * kt + 2,
                                    ct * 128 : (ct + 1) * 128,
                                ],
                                rhs=wv_sb[:, 2 * kt : 2 * kt + 2, :],
                                start=(kt == 0),
                                stop=(kt == 1),
                                perf_mode=DR,
                            )
                        ps_v = ps.rearrange(
                            "p (pr parity d) -> p pr parity d", pr=2, parity=2
                        )
                        prr, kt2 = divmod(ct, 2)
                        nc.vector.tensor_copy(
                            out=vt8[:, prr, kt2, :, 0, :], in_=ps_v[:, :, 0, :]
                        )
                        nc.vector.tensor_copy(
                            out=vt8[:, prr, kt2, :, 3, :], in_=ps_v[:, :, 1, :]
                        )

                # emission order targets the first-scores gate: full k pair
                # 0, the q columns for tq=0, the first v^T chunk pair; the
                # rest streams in behind early attention.
                _qk(2, [0, 1, 2, 3])
                _qk(0, [0])
                _vt([0, 1])
                _qk(0, [1, 2, 3])
                _vt(range(2, NCT))
                _qk(3, [0, 1, 2, 3])
                _qk(1, [0, 1, 2, 3])

            # ---- Phase B: attention per head-pair p, per t-chunk tq ----
            with (
                tc.tile_pool(name="psB", bufs=1, space="PSUM") as psB,
                tc.tile_pool(name="epool", bufs=4) as epool,
                tc.tile_pool(name="rpool", bufs=3) as rpool,
            ):
                pending_norm: list = []
                prev_av = None

                def _av(pr, e8, p, av_e, av_o):
                    # A.V + Z in fp8 DoubleRow: contract both chunks
                    # of the pair in one instruction per head.
                    nc.tensor.matmul(
                        out=av_e[:],
                        lhsT=vt8[:, pr, :, p, 0:2, :],
                        rhs=e8[:, 0, :, :],
                        start=(pr == 0),
                        stop=(pr == NPR - 1),
                        perf_mode=DR,
                    )
                    nc.tensor.matmul(
                        out=av_o[:],
                        lhsT=vt8[:, pr, :, p, 2:4, :],
                        rhs=e8[:, 1, :, :],
                        start=(pr == 0),
                        stop=(pr == NPR - 1),
                        perf_mode=DR,
                    )

                for p in range(2):
                    q_t = qk_sb[:, p, :]
                    k_t = qk_sb[:, 2 + p, :]
                    for tq in range(NQ):
                        ts = slice(tq * 512, (tq + 1) * 512)
                        # fused numerator+denominator accumulators:
                        # av_e rows 0:63 = num_even, rows 64:127 = Z_even
                        # av_o rows 0:63 = Z_odd,   rows 64:127 = num_odd
                        av_e = psB.tile([128, 512], f32, tag="ave", bufs=1)
                        av_o = psB.tile([128, 512], f32, tag="avo", bufs=1)
                        for pr in range(NPR):
                            # E tile for this s-chunk pair: [head, kt, t]
                            e8 = epool.tile([128, 2, 2, 512], fp8, tag="e8")
                            for kt in range(2):
                                ct = 2 * pr + kt
                                cs = slice(ct * 128, (ct + 1) * 128)
                                sc = psB.tile(
                                    [128, 1024], f32, tag="sc", bufs=3
                                )
                                # scores S^T[s, t] for the two heads,
                                # row-packed K=64 at partitions 0 / 64.
                                nc.tensor.matmul(
                                    out=sc[:, 0:512],
                                    lhsT=k_t[0:64, cs],
                                    rhs=q_t[0:64, ts],
                                    start=True,
                                    stop=True,
                                )
                                nc.tensor.matmul(
                                    out=sc[:, 512:1024],
                                    lhsT=k_t[64:128, cs],
                                    rhs=q_t[64:128, ts],
                                    start=True,
                                    stop=True,
                                )
                                # DVE (slower Schraudolph) gets the earlier
                                # chunk, ActE exact exp the later one.
                                e_out = e8[:, :, kt, :]
                                if kt == 0:
                                    nc.vector.tensor_scalar(
                                        out=e_out.bitcast(i8),
                                        in0=sc[:],
                                        scalar1=float(SCH_A8 * SCALE),
                                        scalar2=float(SCH_B8),
                                        op0=mybir.AluOpType.mult,
                                        op1=mybir.AluOpType.add,
                                    )
                                else:
                                    nc.scalar.activation(
                                        out=e_out,
                                        in_=sc[:],
                                        func=Exp,
                                        scale=SCALE,
                                    )
                            # software-pipeline: issue the PREVIOUS pair's
                            # A.V matmuls now (carried across iteration
                            # boundaries), so the tensor engine never waits
                            # on an in-flight exp.
                            if prev_av is not None:
                                _av(*prev_av)
                            prev_av = (pr, e8, p, av_e, av_o)
                            # drain deferred normalize steps from the
                            # previous iteration so the recip chain never
                            # blocks this iteration's exps on ActE.
                            for _ in range(2):
                                if pending_norm:
                                    pending_norm.pop(0)()
                        # normalize: av -> SBUF bf16 (copies split across
                        # ActE/DVE so the av banks recycle fast), gather Z
                        # rows into numerator-aligned partitions (SBUF DMA),
                        # 1/Z = exp(-ln Z) (ActE), scale numerators (GPSIMD,
                        # all-SBUF operands).  The whole chain is deferred
                        # into the next iteration's pair loop; the copies
                        # come first so the av banks free quickly.
                        av_sb = rpool.tile([128, 2, 512], bf16, tag="avsb")
                        z2 = rpool.tile([128, 512], bf16, tag="z2")
                        lnz = rpool.tile([128, 512], f32, tag="lnz")
                        rc = rpool.tile([128, 512], bf16, tag="rc")

                        def _norm_tail(av_sb=av_sb, z2=z2, lnz=lnz, rc=rc,
                                       p=p, ts=ts, av_e=av_e, av_o=av_o):
                            def _cp_gather():
                                nc.scalar.copy(out=av_sb[:, 0, :], in_=av_e)
                                nc.vector.tensor_copy(
                                    out=av_sb[:, 1, :], in_=av_o
                                )
                                nc.gpsimd.dma_start(
                                    out=z2[0:64, :], in_=av_sb[64:128, 0, :]
                                )
                                nc.gpsimd.dma_start(
                                    out=z2[64:128, :], in_=av_sb[0:64, 1, :]
                                )

                            return [
                                _cp_gather,
                                lambda: nc.scalar.activation(
                                    out=lnz, in_=z2, func=Ln
                                ),
                                lambda: nc.scalar.activation(
                                    out=rc, in_=lnz, func=Exp, scale=-1.0
                                ),
                                lambda: nc.gpsimd.tensor_mul(
                                    out=attn_sb[0:64, p, ts],
                                    in0=av_sb[0:64, 0, :],
                                    in1=rc[0:64, :],
                                ),
                                lambda: nc.gpsimd.tensor_mul(
                                    out=attn_sb[64:128, p, ts],
                                    in0=av_sb[64:128, 1, :],
                                    in1=rc[64:128, :],
                                ),
                            ]

                        pending_norm.extend(_norm_tail())
                if prev_av is not None:
                    _av(*prev_av)
                while pending_norm:
                    pending_norm.pop(0)()

            # ---- Phase C: partial projection, t-range-major so each
            # t-range's output DMA starts as soon as it is ready ----
            with (
                tc.tile_pool(name="psC", bufs=1, space="PSUM") as psC,
                tc.tile_pool(name="ypool", bufs=2) as ypool,
            ):
                y_r = y.rearrange("(mt p) t -> p mt t", p=128)
                for nq in range(NQ):
                    for mt in range(4):
                        pj = psC.tile([128, 512], f32, tag="pj", bufs=6)
                        for kt in range(2):
                            nc.tensor.matmul(
                                out=pj,
                                lhsT=wp_sb[:, kt, mt * 128 : (mt + 1) * 128],
                                rhs=attn_sb[
                                    :, kt, nq * 512 : (nq + 1) * 512
                                ],
                                start=(kt == 0),
                                stop=(kt == 1),
                            )
                        y_t = ypool.tile([128, 512], bf16, tag="y", bufs=8)
                        if mt % 2 == 0:
                            nc.vector.tensor_copy(out=y_t, in_=pj)
                            nc.sync.dma_start(
                                out=y_r[:, mt, nq * 512 : (nq + 1) * 512],
                                in_=y_t,
                            )
                        else:
                            nc.scalar.copy(out=y_t, in_=pj)
                            nc.scalar.dma_start(
                                out=y_r[:, mt, nq * 512 : (nq + 1) * 512],
                                in_=y_t,
                            )

    nc.compile()
    return nc


def _get_runner():
    """Build (once) a cached jitted 8-core SPMD executor for the kernel."""
    if "runner" in _CACHE:
        return _CACHE["runner"]

    import jax
    import numpy as _np
    from jax.sharding import Mesh, PartitionSpec
    from jax.experimental.shard_map import shard_map

    from concourse import bass2jax, mybir

    nc = _build_nc()
    bass2jax.install_neuronx_cc_hook()

    partition_name = (
        nc.partition_id_tensor.name if nc.partition_id_tensor else None
    )
    in_names: list[str] = []
    out_names: list[str] = []
    out_avals = []
    zero_outs: list[_np.ndarray] = []
    for alloc in nc.m.functions[0].allocations:
        if not isinstance(alloc, mybir.MemoryLocationSet):
            continue
        name = alloc.memorylocations[0].name
        if alloc.kind == "ExternalInput":
            if name != partition_name:
                in_names.append(name)
        elif alloc.kind == "ExternalOutput":
            shape = tuple(alloc.tensor_shape)
            dtype = mybir.dt.np(alloc.dtype)
            out_names.append(name)
            out_avals.append(jax.core.ShapedArray(shape, dtype))
            zero_outs.append(_np.zeros(shape, dtype))
    n_params = len(in_names)
    n_outs = len(out_avals)
    in_names_all = in_names + out_names
    if partition_name is not None:
        in_names_all.append(partition_name)

    donate = tuple(range(n_params, n_params + n_outs))

    def _body(*args):
        operands = list(args)
        if partition_name is not None:
            operands.append(bass2jax.partition_id_tensor())
        outs = bass2jax._bass_exec_p.bind(
            *operands,
            out_avals=tuple(out_avals),
            in_names=tuple(in_names_all),
            out_names=tuple(out_names),
            lowering_input_output_aliases=(),
            sim_require_finite=True,
            sim_require_nnan=True,
            nc=nc,
        )
        return tuple(outs)

    devices = jax.devices()[:N_CORES]
    mesh = Mesh(np.asarray(devices), ("core",))
    in_specs = (PartitionSpec("core"),) * (n_params + n_outs)
    out_specs = (PartitionSpec("core"),) * n_outs
    sharded = jax.jit(
        shard_map(
            _body, mesh=mesh, in_specs=in_specs, out_specs=out_specs,
            check_rep=False,
        ),
        donate_argnums=donate,
        keep_unused=True,
    )

    runner = {
        "fn": sharded,
        "in_names": in_names,
        "out_names": out_names,
        "zero_outs": zero_outs,
    }
    _CACHE["runner"] = runner
    return runner


def _prepare_in_maps(x, qkv_w, qkv_b, proj_w, proj_b):
    """Full inputs -> per-core input dicts (batch x head-group sharding)."""
    in_maps = []
    for c in range(N_CORES):
        b, g = divmod(c, G)
        qs = slice(g * CG, (g + 1) * CG)
        wq = qkv_w[qs, :]
        wk = qkv_w[C + g * CG : C + (g + 1) * CG, :]
        wv = qkv_w[2 * C + g * CG : 2 * C + (g + 1) * CG, :]
        wqkT = np.ascontiguousarray(np.concatenate([wq, wk], axis=0).T)
        wvT = np.ascontiguousarray(wv.T)
        wpT = np.ascontiguousarray(proj_w[:, qs].T)
        bqk = np.ascontiguousarray(
            np.concatenate([qkv_b[qs], qkv_b[C + g * CG : C + (g + 1) * CG]])
        ).reshape(2 * CG, 1)
        in_maps.append(
            {
                "x": np.ascontiguousarray(x[b]).astype(_FP8),
                "wqkT": wqkT.astype(_FP8),
                "wvT": wvT.astype(_FP8),
                "wpT": wpT.astype(_BF16),
                "bqk": bqk.astype(np.float32),
            }
        )
    return in_maps


def _run_in_maps(in_maps):
    """Run the SPMD kernel, return list of per-core output dicts."""
    r = _get_runner()
    per_core = [
        [np.asarray(m[name]) for name in r["in_names"]] for m in in_maps
    ]
    concat_in = [
        np.concatenate([per_core[c][i] for c in range(N_CORES)], axis=0)
        for i in range(len(r["in_names"]))
    ]
    concat_zero = [
        np.concatenate([z] * N_CORES, axis=0) for z in r["zero_outs"]
    ]
    outs = r["fn"](*concat_in, *concat_zero)
    outs = [np.asarray(o) for o in outs]
    results = []
    for c in range(N_CORES):
        d = {}
        for i, name in enumerate(r["out_names"]):
            per_len = outs[i].shape[0] // N_CORES
            d[name] = outs[i][c * per_len : (c + 1) * per_len]
        results.append(d)
    return results


def kernel(x, qkv_w, qkv_b, proj_w, proj_b):
    x = np.asarray(x, dtype=np.float32)
    qkv_w = np.asarray(qkv_w, dtype=np.float32)
    qkv_b = np.asarray(qkv_b, dtype=np.float32)
    proj_w = np.asarray(proj_w, dtype=np.float32)
    proj_b = np.asarray(proj_b, dtype=np.float32)

    in_maps = _prepare_in_maps(x, qkv_w, qkv_b, proj_w, proj_b)
    results = _run_in_maps(in_maps)

    # host combine: residual + effective projection bias + the two
    # head-group partials per batch.
    bp_eff = proj_b + proj_w @ qkv_b[2 * C : 3 * C]
    out = np.empty((B, C, T), dtype=np.float32)
    for b in range(B):
        out[b] = (
            x[b]
            + bp_eff[:, None]
            + results[G * b]["y"]
            + results[G * b + 1]["y"]
        )
    return out
